# revision 5
# baseline (speedup 1.0000x reference)
"""Trainium2 Bass kernel for nn_PokerTransformerV1 (ragged-cache attention).

Strategy (batch data-parallel, one sample per NeuronCore):
  - Host specializes on the runtime past_lengths/new_lengths: past KV is
    processed in ceil(max(past_len)/128) chunks of 128 keys; invalid keys are
    masked via an additive -30 bias folded into the Exp activation.
  - Scores are computed TRANSPOSED (s^T[j, t]) so the attention-prob matrix
    comes out with keys on partitions, ready to be the contraction operand of
    the P^T @ V matmul without any transpose of the probabilities.
  - The softmax denominator is obtained for free by augmenting V with a ones
    column (row 64 of the ctx psum accumulator = rowsum of probabilities).
  - Normalization happens on the ctx^T tiles (which are exactly the lhsT
    chunks the output projection needs).
  - RoPE is applied in the natural [token, dmodel] layout with host-precomputed
    cos/sin tables; q/k are transposed per head-pair on the PE.
  - Matmul dtype: float32r (tf32-like, ~1.5e-4 rel err) everywhere except the
    P @ V contraction, which runs in bf16 (probabilities are positive and
    well-conditioned there).
"""

import numpy as np
import ml_dtypes

B, T, DM, H, DH = 8, 128, 1024, 16, 64
NPAIR = H // 2
NEG = -30.0
N_CORES = 8

_CACHE = {}


def _build(nch, qkv_bias, o_bias, dyn_reps=False):
    import concourse.bacc as bacc
    import concourse.mybir as mybir
    import concourse.tile as tile

    dt = mybir.dt
    AF = mybir.ActivationFunctionType
    OP = mybir.AluOpType

    nc = bacc.Bacc("TRN2", target_bir_lowering=False, debug=False,
                   num_devices=N_CORES)

    D_xT = nc.dram_tensor("xT", [128, 8 * 128], dt.float32r, kind="ExternalInput")
    D_wq = nc.dram_tensor("WqT", [8, 128, 1024], dt.float32r, kind="ExternalInput")
    D_wk = nc.dram_tensor("WkT", [8, 128, 1024], dt.float32r, kind="ExternalInput")
    D_wv = nc.dram_tensor("WvT", [8, 128, 1024], dt.float32r, kind="ExternalInput")
    D_wo = nc.dram_tensor("WoH", [16, 64, 1024], dt.float32r, kind="ExternalInput")
    D_kt = nc.dram_tensor("KT", [128, nch * 8 * 128], dt.float32r, kind="ExternalInput")
    D_va = nc.dram_tensor("VA", [128, nch * 16 * 65], dt.bfloat16, kind="ExternalInput")
    D_cos = nc.dram_tensor("COS", [128, 64], dt.float32, kind="ExternalInput")
    D_sinN = nc.dram_tensor("SINN", [128, 32], dt.float32, kind="ExternalInput")
    D_sinP = nc.dram_tensor("SINP", [128, 32], dt.float32, kind="ExternalInput")
    D_mask = nc.dram_tensor("MASK", [128, nch + 1], dt.float32, kind="ExternalInput")
    D_valid = nc.dram_tensor("VALID", [128, 1], dt.float32, kind="ExternalInput")
    D_id = nc.dram_tensor("IDENT", [128, 128], dt.float32r, kind="ExternalInput")
    D_bqkv = None
    if qkv_bias:
        D_bqkv = nc.dram_tensor("BQKV", [3, 128, 1024], dt.float32, kind="ExternalInput")
    D_bo = None
    if o_bias:
        D_bo = nc.dram_tensor("BO", [128, 1024], dt.float32, kind="ExternalInput")
    D_reps = None
    if dyn_reps:
        D_reps = nc.dram_tensor("REPS", [1, 1], dt.int32, kind="ExternalInput")
    D_out = nc.dram_tensor("OUT", [128, 1024], dt.float32, kind="ExternalOutput")

    with tile.TileContext(nc) as tc:

        def body(cpool, wpool, dvepool, ptpool, rbpool):
            # ---- resident loads -------------------------------------------------
            kt = cpool.tile([128, nch * 8, 128], dt.float32r, tag="kt")
            va = cpool.tile([128, nch, 16, 65], dt.bfloat16, tag="va")
            xT = cpool.tile([128, 8, 128], dt.float32r, tag="xT")
            cos = cpool.tile([128, 64], dt.float32, tag="cos")
            sinN = cpool.tile([128, 32], dt.float32, tag="sinN")
            sinP = cpool.tile([128, 32], dt.float32, tag="sinP")
            mask = cpool.tile([128, nch + 1], dt.float32, tag="mask")
            valid = cpool.tile([128, 1], dt.float32, tag="valid")
            ident = cpool.tile([128, 128], dt.float32r, tag="ident")
            nc.sync.dma_start(out=kt[:].rearrange("p c j -> p (c j)"), in_=D_kt.ap())
            nc.sync.dma_start(out=va[:].rearrange("p c h e -> p (c h e)"), in_=D_va.ap())
            nc.sync.dma_start(out=xT[:].rearrange("p c t -> p (c t)"), in_=D_xT.ap())
            nc.sync.dma_start(out=cos[:], in_=D_cos.ap())
            nc.sync.dma_start(out=sinN[:], in_=D_sinN.ap())
            nc.sync.dma_start(out=sinP[:], in_=D_sinP.ap())
            nc.sync.dma_start(out=mask[:], in_=D_mask.ap())
            nc.sync.dma_start(out=valid[:], in_=D_valid.ap())
            nc.sync.dma_start(out=ident[:], in_=D_id.ap())
            bqkv = None
            if qkv_bias:
                bqkv = cpool.tile([128, 3, 1024], dt.float32, tag="bqkv")
                nc.sync.dma_start(out=bqkv[:].rearrange("p a n -> p (a n)"),
                                  in_=D_bqkv.ap().rearrange("a p n -> p (a n)"))
            bo = None
            if o_bias:
                bo = cpool.tile([128, 1024], dt.float32, tag="bo")
                nc.sync.dma_start(out=bo[:], in_=D_bo.ap())

            q_rope = dvepool.tile([128, 1024], dt.float32r, tag="q_rope")
            k_rope = dvepool.tile([128, 1024], dt.float32r, tag="k_rope")
            va_new = dvepool.tile([128, 16, 65], dt.bfloat16, tag="va_new")
            qTbd = dvepool.tile([128, 8 * 256], dt.float32r, tag="qTbd")
            kTn = dvepool.tile([128, 8 * 128], dt.float32r, tag="kTn")
            ctxT = dvepool.tile([64, 16 * 128], dt.float32r, tag="ctxT")
            rsum = dvepool.tile([1, 2048], dt.float32, tag="rsum")
            out_sb = dvepool.tile([128, 1024], dt.float32, tag="out_sb")

            cosb = cos[:, None, :].broadcast_to([128, 16, 64])
            sinNb = sinN[:, None, :].broadcast_to([128, 16, 32])
            sinPb = sinP[:, None, :].broadcast_to([128, 16, 32])

            def rope(src_ap, dst, scratch_tag):
                # dst = src*cos + rotate_half(src)*sin   (src in [t, h*64+d])
                qc = dvepool.tile([128, 1024], dt.float32, tag=scratch_tag + "_qc")
                t1 = dvepool.tile([128, 1024], dt.float32, tag=scratch_tag + "_t1")
                sv = src_ap.rearrange("p (h s r) -> p h s r", h=16, s=2)
                t1v = t1[:].rearrange("p (h s r) -> p h s r", h=16, s=2)
                nc.vector.tensor_tensor(
                    out=qc[:].rearrange("p (h d) -> p h d", h=16),
                    in0=src_ap.rearrange("p (h d) -> p h d", h=16),
                    in1=cosb, op=OP.mult)
                nc.vector.tensor_tensor(out=t1v[:, :, 0, :], in0=sv[:, :, 1, :],
                                        in1=sinNb, op=OP.mult)
                nc.vector.tensor_tensor(out=t1v[:, :, 1, :], in0=sv[:, :, 0, :],
                                        in1=sinPb, op=OP.mult)
                nc.vector.tensor_tensor(out=dst[:], in0=qc[:], in1=t1[:], op=OP.add)

            # ---- projections + RoPE + transposes -------------------------------
            with (
                tc.tile_pool(name="pp", bufs=2, space="PSUM") as pp,
                tc.tile_pool(name="tp", bufs=3, space="PSUM") as tp,
            ):
                for wi, wdram in enumerate((D_wq, D_wk, D_wv)):
                    ps = pp.tile([128, 1024], dt.float32, tag="pp")
                    for mc in range(8):
                        wt = wpool.tile([128, 1024], dt.float32r, tag="wproj")
                        nc.sync.dma_start(out=wt[:], in_=wdram.ap()[mc])
                        nc.tensor.matmul(ps[:, 0:512], xT[:, mc, :], wt[:, 0:512],
                                         start=(mc == 0), stop=(mc == 7))
                        nc.tensor.matmul(ps[:, 512:1024], xT[:, mc, :], wt[:, 512:1024],
                                         start=(mc == 0), stop=(mc == 7))
                    src = ps[:]
                    if qkv_bias:
                        st = dvepool.tile([128, 1024], dt.float32, tag="bias_st")
                        nc.vector.tensor_tensor(out=st[:], in0=ps[:],
                                                in1=bqkv[:, wi, :], op=OP.add)
                        src = st[:]
                    if wi == 0:
                        rope(src, q_rope, "rp")
                    elif wi == 1:
                        rope(src, k_rope, "rp")
                    else:
                        nc.vector.tensor_copy(
                            va_new[:, :, 0:64],
                            src.rearrange("p (h d) -> p h d", h=16))
                        nc.vector.tensor_scalar_mul(
                            va_new[:, :, 64:65],
                            valid[:, None, :].broadcast_to([128, 16, 1]), 1.0)

                nc.vector.tensor_scalar_mul(qTbd[:], qTbd[:], 0.0)
                for p in range(NPAIR):
                    tq = tp.tile([128, 128], dt.float32r, tag="tp")
                    nc.tensor.transpose(tq[:], q_rope[:, p * 128:(p + 1) * 128], ident[:])
                    nc.vector.tensor_copy(qTbd[0:64, p * 256:p * 256 + 128], tq[0:64, :])
                    nc.vector.tensor_copy(qTbd[64:128, p * 256 + 128:p * 256 + 256],
                                          tq[64:128, :])
                    tk = tp.tile([128, 128], dt.float32r, tag="tp")
                    nc.tensor.transpose(tk[:], k_rope[:, p * 128:(p + 1) * 128], ident[:])
                    nc.vector.tensor_copy(kTn[:, p * 128:(p + 1) * 128], tk[:])

            # ---- attention ------------------------------------------------------
            with tc.tile_pool(name="cx", bufs=1, space="PSUM") as cxp:
                cx = []
                for g in range(4):
                    cxt = cxp.tile([65, 512], dt.float32, tag=f"cx{g}")
                    nc.vector.memset(cxt[:], 0.0)
                    cx.append(cxt)
                with tc.tile_pool(name="sc", bufs=2, space="PSUM") as scp:
                    for c in range(nch + 1):
                        for half in range(2):
                            ss = scp.tile([128, 1024], dt.float32, tag="sc")
                            for i in range(4):
                                p = half * 4 + i
                                if c < nch:
                                    lhs = kt[:, c * 8 + p, :]
                                else:
                                    lhs = kTn[:, p * 128:(p + 1) * 128]
                                nc.tensor.matmul(ss[:, i * 256:(i + 1) * 256], lhs,
                                                 qTbd[:, p * 256:(p + 1) * 256],
                                                 start=True, stop=True)
                            pt = ptpool.tile([128, 1024], dt.bfloat16, tag="pt")
                            nc.scalar.activation(pt[:], ss[:], AF.Exp,
                                                 bias=mask[:, c:c + 1], scale=0.125)
                            for i in range(4):
                                p = half * 4 + i
                                for h2 in range(2):
                                    h = 2 * p + h2
                                    if c < nch:
                                        va_s = va[:, c, h, :]
                                    else:
                                        va_s = va_new[:, h, :]
                                    g, col = h // 4, (h % 4) * 128
                                    nc.tensor.matmul(
                                        cx[g][0:65, col:col + 128], va_s,
                                        pt[:, i * 256 + h2 * 128:i * 256 + (h2 + 1) * 128],
                                        start=False, stop=(c == nch and h % 4 == 3))

                # ---- normalize into ctxT (the out-proj lhsT chunks) -------------
                for g in range(4):
                    nc.vector.reciprocal(rsum[0:1, g * 512:(g + 1) * 512],
                                         cx[g][64:65, :])
                for h in range(16):
                    g, col = h // 4, (h % 4) * 128
                    rbt = rbpool.tile([64, 128], dt.float32, tag="rb")
                    nc.gpsimd.partition_broadcast(
                        rbt[:], rsum[0:1, g * 512 + col:g * 512 + col + 128])
                    nc.vector.tensor_tensor(out=ctxT[:, h * 128:(h + 1) * 128],
                                            in0=cx[g][0:64, col:col + 128],
                                            in1=rbt[:], op=OP.mult)

            # ---- output projection ---------------------------------------------
            with tc.tile_pool(name="po", bufs=1, space="PSUM") as pop:
                po = pop.tile([128, 1024], dt.float32, tag="po")
                for h in range(16):
                    wo = wpool.tile([64, 1024], dt.float32r, tag="wo")
                    nc.sync.dma_start(out=wo[:], in_=D_wo.ap()[h])
                    nc.tensor.matmul(po[:, 0:512], ctxT[:, h * 128:(h + 1) * 128],
                                     wo[:, 0:512], start=(h == 0), stop=(h == 15))
                    nc.tensor.matmul(po[:, 512:1024], ctxT[:, h * 128:(h + 1) * 128],
                                     wo[:, 512:1024], start=(h == 0), stop=(h == 15))
                if o_bias:
                    nc.vector.tensor_tensor(out=out_sb[:], in0=po[:], in1=bo[:],
                                            op=OP.add)
                    nc.vector.tensor_scalar_mul(out_sb[:], out_sb[:], valid[:, 0:1])
                else:
                    nc.vector.tensor_scalar_mul(out_sb[:], po[:], valid[:, 0:1])
            nc.sync.dma_start(out=D_out.ap(), in_=out_sb[:])

        with (
            tc.tile_pool(name="const", bufs=1) as cpool,
            tc.tile_pool(name="w", bufs=3) as wpool,
            tc.tile_pool(name="dve", bufs=1) as dvepool,
            tc.tile_pool(name="ptp", bufs=3) as ptpool,
            tc.tile_pool(name="rbp", bufs=2) as rbpool,
        ):
            if dyn_reps:
                rt = cpool.tile([1, 1], dt.int32, tag="rt")
                nc.sync.dma_start(out=rt[:], in_=D_reps.ap())
                regs = nc.alloc_registers()
                nc.regs_load(regs, rt[0:1, 0:1])
                rv = nc.snap(regs, donate=True, min_val=1, max_val=100000)
                with tc.For_i(0, rv, 1):
                    body(cpool, wpool, dvepool, ptpool, rbpool)
            else:
                body(cpool, wpool, dvepool, ptpool, rbpool)

    nc.finalize()
    return nc


def _prep_host(inputs):
    """Build per-core input maps from the full-batch inputs."""
    x = np.ascontiguousarray(np.asarray(inputs["x_new"], dtype=np.float32))
    pk = np.asarray(inputs["past_k"], dtype=np.float32)
    pv = np.asarray(inputs["past_v"], dtype=np.float32)
    Wq = np.asarray(inputs["Wq"], dtype=np.float32)
    Wk = np.asarray(inputs["Wk"], dtype=np.float32)
    Wv = np.asarray(inputs["Wv"], dtype=np.float32)
    Wo = np.asarray(inputs["Wo"], dtype=np.float32)
    bq = np.asarray(inputs["bq"], dtype=np.float32)
    bk = np.asarray(inputs["bk"], dtype=np.float32)
    bv = np.asarray(inputs["bv"], dtype=np.float32)
    bo = np.asarray(inputs["bo"], dtype=np.float32)
    inv_freq = np.asarray(inputs["inv_freq"], dtype=np.float32)
    pl = np.asarray(inputs["past_lengths"]).astype(np.int64)
    nl = np.asarray(inputs["new_lengths"]).astype(np.int64)

    nch = max(1, int(-(-int(pl.max()) // 128)))
    qkv_bias = bool(np.any(bq) or np.any(bk) or np.any(bv))
    o_bias = bool(np.any(bo))

    WqT = np.ascontiguousarray(Wq.T.reshape(8, 128, 1024))
    WkT = np.ascontiguousarray(Wk.T.reshape(8, 128, 1024))
    WvT = np.ascontiguousarray(Wv.T.reshape(8, 128, 1024))
    WoH = np.ascontiguousarray(Wo.T.reshape(16, 64, 1024))
    ident = np.eye(128, dtype=np.float32)
    steps = np.arange(T)

    shared = {"WqT": WqT, "WkT": WkT, "WvT": WvT, "WoH": WoH, "IDENT": ident}
    if qkv_bias:
        shared["BQKV"] = np.ascontiguousarray(
            np.broadcast_to(np.stack([bq, bk, bv])[:, None, :], (3, 128, 1024)))
    if o_bias:
        shared["BO"] = np.ascontiguousarray(np.broadcast_to(bo[None, :], (128, 1024)))

    in_maps = []
    for b in range(B):
        plb, nlb = int(pl[b]), int(nl[b])
        S = nch * 128
        # xT image: [m-part 128, (mc, t)]
        xT = np.ascontiguousarray(
            x[b].T.reshape(8, 128, 128).transpose(1, 0, 2).reshape(128, 1024))
        # KT image: [dpair 128, (c, pair, j)]
        KT = np.ascontiguousarray(
            pk[b, :, :S, :].reshape(8, 2, nch, 128, 64)
            .transpose(1, 4, 2, 0, 3).reshape(128, nch * 8 * 128))
        # VA image: [j 128, (c, h, 65)]
        vparts = pv[b, :, :S, :].transpose(1, 0, 2).reshape(nch, 128, 16, 64)
        vparts = vparts.transpose(1, 0, 2, 3)  # [j, c, h, d]
        jj = (np.arange(nch)[None, :] * 128 + np.arange(128)[:, None])  # [j, c]
        ones = (jj < plb).astype(np.float32)[:, :, None, None]
        ones = np.broadcast_to(ones, (128, nch, 16, 1))
        VA = np.concatenate([vparts, ones], axis=-1).astype(ml_dtypes.bfloat16)
        VA = np.ascontiguousarray(VA.reshape(128, nch * 16 * 65))
        # RoPE tables
        pos = (plb + steps).astype(np.float32)
        ang = pos[:, None] * inv_freq[None, :]  # [128, 32]
        COS = np.ascontiguousarray(
            np.concatenate([np.cos(ang), np.cos(ang)], axis=1).astype(np.float32))
        SINN = np.ascontiguousarray((-np.sin(ang)).astype(np.float32))
        SINP = np.ascontiguousarray(np.sin(ang).astype(np.float32))
        # masks
        MASK = np.full((128, nch + 1), NEG, dtype=np.float32)
        MASK[:, :nch][jj < plb] = 0.0
        MASK[steps < nlb, nch] = 0.0
        VALID = (steps < nlb).astype(np.float32)[:, None]
        m = {"xT": xT, "KT": KT, "VA": VA, "COS": COS, "SINN": SINN,
             "SINP": SINP, "MASK": np.ascontiguousarray(MASK),
             "VALID": np.ascontiguousarray(VALID)}
        m.update(shared)
        in_maps.append(m)
    return in_maps, nch, qkv_bias, o_bias


def kernel(**inputs) -> np.ndarray:
    from concourse.bass_utils import run_bass_kernel_spmd

    in_maps, nch, qkv_bias, o_bias = _prep_host(inputs)
    key = (nch, qkv_bias, o_bias, False)
    if key not in _CACHE:
        _CACHE[key] = _build(nch, qkv_bias, o_bias, dyn_reps=False)
    nc = _CACHE[key]
    res = run_bass_kernel_spmd(nc, in_maps, list(range(N_CORES)))
    out = np.stack([res.results[b]["OUT"] for b in range(B)], axis=0)
    return out.astype(np.float32)
